# revision 1
# baseline (speedup 1.0000x reference)
"""EAST-style loss (weighted BCE score + smoothed-L1 geometry) on 8 trn2 cores.

Strategy: pure data parallel over batch m=128 -> 16 per core. Each core streams
its shard through SBUF once (memory-bound), computing per-partition partial sums
with fused accumulate ops:
  DVE: d = yt-yp, min(|d|,1) (tensor_scalar abs_max+min), relu(d-1) w/ accum,
       yt*ln(yp) / yt*ln(1-yp) via scalar_tensor_tensor w/ accum, sum(yt)
  ACT: square(min) w/ accum, relu(-d-1) w/ accum, ln(yp), ln(1-yp) w/ accum
Final scalar combine happens on host in float64 (stats are tiny: [128, ~17]).

huber identity used: for l1=|d|:  huber(l1) = 0.5*min(l1,1)^2 + relu(l1-1)
and relu(l1-1) = relu(d-1) + relu(-d-1)  (at most one side nonzero).
"""

import sys

sys.path.insert(0, "/opt/trn_rl_repo")

import numpy as np

import concourse.bacc as bacc
import concourse.mybir as mybir
from concourse.bass_utils import run_bass_kernel_spmd
from concourse.tile import TileContext

N_CORES = 8
M, H, W = 128, 128, 128
GC = 8  # geometry channels
M_PER = M // N_CORES  # 16

P = 128
F = 2048
SCORE_ELEMS = M_PER * 1 * H * W  # 262144 = P*F exactly
GEOM_ELEMS = M_PER * GC * H * W  # 2097152
N_GT = GEOM_ELEMS // (P * F)  # 8 geometry tiles per core

# stats_act columns: [0:8]=sum(min^2) per tile, [8:16]=sum(relu(-d-1)), [16]=sum(ln(1-yp))
NS_ACT = 2 * N_GT + 1
# stats_dve columns: [0:8]=sum(relu(d-1)), [8]=sum(yt*ln yp), [9]=sum(yt*ln(1-yp)), [10]=sum(yt)
NS_DVE = N_GT + 3

F32 = mybir.dt.float32

_CACHED_NC = None


def _build_nc():
    nc = bacc.Bacc("TRN2", target_bir_lowering=False)
    f32 = F32
    yt_s = nc.dram_tensor("yt_s", [P, F], f32, kind="ExternalInput")
    yp_s = nc.dram_tensor("yp_s", [P, F], f32, kind="ExternalInput")
    yt_g = nc.dram_tensor("yt_g", [N_GT, P, F], f32, kind="ExternalInput")
    yp_g = nc.dram_tensor("yp_g", [N_GT, P, F], f32, kind="ExternalInput")
    stats_act_d = nc.dram_tensor("stats_act", [P, NS_ACT], f32, kind="ExternalOutput")
    stats_dve_d = nc.dram_tensor("stats_dve", [P, NS_DVE], f32, kind="ExternalOutput")

    AF = mybir.ActivationFunctionType
    OP = mybir.AluOpType

    with TileContext(nc) as tc:
        with (
            tc.tile_pool(name="stats", bufs=1) as spool,
            tc.tile_pool(name="score", bufs=1) as scpool,
            tc.tile_pool(name="io", bufs=4) as iopool,
            tc.tile_pool(name="work", bufs=3) as wpool,
        ):
            st_act = spool.tile([P, NS_ACT], f32)
            st_dve = spool.tile([P, NS_DVE], f32)
            cm1 = spool.tile([P, 1], f32)  # bias constant -1.0 for Relu(-d-1)
            nc.vector.memset(cm1[:], -1.0)

            # ---------------- score part (1 tile pair) ----------------
            yt = scpool.tile([P, F], f32)
            nc.sync.dma_start(out=yt[:], in_=yt_s[:])
            yp = scpool.tile([P, F], f32)
            nc.sync.dma_start(out=yp[:], in_=yp_s[:])
            lnp = scpool.tile([P, F], f32)
            scr = wpool.tile([P, F], f32, tag="scr")
            # ln(yp)
            nc.scalar.activation(lnp[:], yp[:], AF.Ln)
            # ln(1-yp) in-place over yp; accum -> sum(ln(1-yp))
            nc.scalar.activation(
                yp[:], yp[:], AF.Ln, scale=-1.0, bias=1.0,
                accum_out=st_act[:, 2 * N_GT : 2 * N_GT + 1],
            )
            # sum(yt) first on DVE: absorbs the yt-DMA wait so the STT ops
            # below (limited sync-wait slots in the S2S2D2_STT struct) only
            # need a single ACT wait each.
            scr3 = wpool.tile([P, F], f32, tag="scr")
            nc.vector.tensor_scalar(
                out=scr3[:], in0=yt[:], scalar1=1.0, scalar2=0.0,
                op0=OP.mult, op1=OP.add,
                accum_out=st_dve[:, N_GT + 2 : N_GT + 3],
            )
            # sum(yt * ln(yp))  (TTR hangs HW; STT accum works)
            nc.vector.scalar_tensor_tensor(
                out=scr[:], in0=yt[:], scalar=1.0, in1=lnp[:],
                op0=OP.mult, op1=OP.mult,
                accum_out=st_dve[:, N_GT : N_GT + 1],
            )
            scr2 = wpool.tile([P, F], f32, tag="scr")
            # sum(yt * ln(1-yp))
            nc.vector.scalar_tensor_tensor(
                out=scr2[:], in0=yt[:], scalar=1.0, in1=yp[:],
                op0=OP.mult, op1=OP.mult,
                accum_out=st_dve[:, N_GT + 1 : N_GT + 2],
            )

            # ---------------- geometry part (N_GT tile pairs) ----------------
            for i in range(N_GT):
                a = iopool.tile([P, F], f32, tag="a")
                nc.sync.dma_start(out=a[:], in_=yt_g[i])
                b = iopool.tile([P, F], f32, tag="b")
                nc.sync.dma_start(out=b[:], in_=yp_g[i])
                d = wpool.tile([P, F], f32, tag="d")
                nc.vector.tensor_sub(d[:], a[:], b[:])
                # clamp(d,-1,1); its square equals min(|d|,1)^2
                minv = wpool.tile([P, F], f32, tag="minv")
                nc.vector.tensor_scalar(
                    out=minv[:], in0=d[:], scalar1=1.0, scalar2=-1.0,
                    op0=OP.min, op1=OP.max,
                )
                # relu(d-1): (d + -1) max 0, accum; write over b (dead after sub)
                nc.vector.tensor_scalar(
                    out=b[:], in0=d[:], scalar1=-1.0, scalar2=0.0,
                    op0=OP.add, op1=OP.max,
                    accum_out=st_dve[:, i : i + 1],
                )
                # square(min) in-place, accum
                nc.scalar.activation(
                    minv[:], minv[:], AF.Square,
                    accum_out=st_act[:, i : i + 1],
                )
                # relu(-d-1), accum; write over a (dead after sub)
                nc.scalar.activation(
                    a[:], d[:], AF.Relu, scale=-1.0, bias=cm1[:],
                    accum_out=st_act[:, N_GT + i : N_GT + i + 1],
                )

            nc.sync.dma_start(out=stats_act_d[:], in_=st_act[:])
            nc.sync.dma_start(out=stats_dve_d[:], in_=st_dve[:])
    nc.finalize()
    return nc


def _get_nc():
    global _CACHED_NC
    if _CACHED_NC is None:
        _CACHED_NC = _build_nc()
    return _CACHED_NC


def _make_in_maps(Y_true_score, Y_pred_score, Y_true_geometry, Y_pred_geometry):
    yts = np.ascontiguousarray(np.asarray(Y_true_score, dtype=np.float32))
    yps = np.ascontiguousarray(np.asarray(Y_pred_score, dtype=np.float32))
    ytg = np.ascontiguousarray(np.asarray(Y_true_geometry, dtype=np.float32))
    ypg = np.ascontiguousarray(np.asarray(Y_pred_geometry, dtype=np.float32))
    in_maps = []
    for k in range(N_CORES):
        sl = slice(k * M_PER, (k + 1) * M_PER)
        in_maps.append(
            {
                "yt_s": yts[sl].reshape(P, F),
                "yp_s": yps[sl].reshape(P, F),
                "yt_g": ytg[sl].reshape(N_GT, P, F),
                "yp_g": ypg[sl].reshape(N_GT, P, F),
            }
        )
    return in_maps


def _combine(results):
    """results: list of per-core dicts with stats_act [P,NS_ACT], stats_dve [P,NS_DVE]."""
    sq_sum = 0.0
    r1_sum = 0.0
    r2_sum = 0.0
    ln1m_sum = 0.0
    t1_sum = 0.0
    t2_sum = 0.0
    yt_sum = 0.0
    for r in results:
        sa = np.asarray(r["stats_act"], dtype=np.float64)
        sd = np.asarray(r["stats_dve"], dtype=np.float64)
        sq_sum += sa[:, 0:N_GT].sum()
        r2_sum += sa[:, N_GT : 2 * N_GT].sum()
        ln1m_sum += sa[:, 2 * N_GT].sum()
        r1_sum += sd[:, 0:N_GT].sum()
        t1_sum += sd[:, N_GT].sum()
        t2_sum += sd[:, N_GT + 1].sum()
        yt_sum += sd[:, N_GT + 2].sum()

    size = float(M * 1 * H * W)
    beta = 1.0 - yt_sum / size
    A = t1_sum  # sum(yt * ln yp)
    B = ln1m_sum - t2_sum  # sum((1-yt) * ln(1-yp))
    loss_score = (-beta * A - (1.0 - beta) * B) / M

    huber_sum = 0.5 * sq_sum + r1_sum + r2_sum
    n_pix = M * H * W
    loss_geom = huber_sum / GC / n_pix  # LAMBDA_GEOMETRY = 1.0

    return np.array(loss_score + loss_geom, dtype=np.float32)


def kernel(Y_true_score, Y_pred_score, Y_true_geometry, Y_pred_geometry, **_kw):
    nc = _get_nc()
    in_maps = _make_in_maps(
        Y_true_score, Y_pred_score, Y_true_geometry, Y_pred_geometry
    )
    res = run_bass_kernel_spmd(nc, in_maps, core_ids=list(range(N_CORES)))
    return _combine(res.results)



# revision 5
# speedup vs baseline: 1.6431x; 1.6431x over previous
"""EAST-style loss (weighted BCE score + smoothed-L1 geometry) on 8 trn2 cores.

Strategy: pure data parallel over batch m=128 -> 16 per core. Host packs each
core's shard into fp16 (halves HBM traffic; rel-err budget 2e-2 vs ~1e-4 fp16
quantization noise). Each core streams 9MB through SBUF (memory-bound):
  xg [4,128,8192]: geometry pair-tiles, cols 0:4096 = yt chunk, 4096: = yp
  xs [128,4096]:   score,          cols 0:2048 = yt_s,  2048: = yp_s

Geometry uses ONE fused custom-DVE op per pair-tile (registered via the
documented dve_ops extension point): with d = a-b, c = clamp(d,-1,1),
  huber(d) = d*c - 0.5*c^2       (= 0.5 d^2 inside, |d|-0.5 outside)
summed across the free dim by the op's accumulator -> zero ACT work for
geometry, one DVE pass per element. Score: clamp yp below 1.0 (fp16 rounds
1-1e-4 up to 1.0 -> ln(0)), ln/ln(1-.) on ACT with accum, yt*ln products on
DVE with accum. Final scalar combine happens on host in float64 (stats are
tiny: [128, 8]).
"""

import sys

sys.path.insert(0, "/opt/trn_rl_repo")

import numpy as np

import concourse.bacc as bacc
import concourse.mybir as mybir
from concourse.bass_utils import run_bass_kernel_spmd
from concourse.tile import TileContext

N_CORES = 8
M, H, W = 128, 128, 128
GC = 8  # geometry channels
M_PER = M // N_CORES  # 16

P = 128
FG = 4096  # geometry chunk free-dim per pair-tile half (fp16)
N_GT = 4  # geometry pair-tiles per core: 4 * 128 * 4096 = 2,097,152 elems
FS = 2048  # score free-dim per half (fp16)

# fp16-representable clamp just below 1.0 so ln(1-yp) stays finite
YP_MAX = 0.99951171875

# stats columns (single fp32 [P, 8] tensor):
#   [0:4] = sum huber(d) per geometry tile   (custom DVE accum)
#   [4]   = sum(ln(1-yp))                    (ACT accum)
#   [5]   = sum(yt_s)                        (ACT accum)
#   [6]   = sum(yt_s * ln(yp))               (DVE accum)
#   [7]   = sum(yt_s * ln(1-yp))             (DVE accum)
NS = 8

F16 = mybir.dt.float16
F32 = mybir.dt.float32

_CACHED_NC = None
_HUBER_OP = None


def _register_huber_op():
    """Register the fused huber+accumulate custom-DVE op (idempotent).

    Uses the documented dve_ops extension point (04-custom-dve-api.md): the
    op's uop program is written into the per-NEFF DVE table at compile time.
    """
    global _HUBER_OP
    if _HUBER_OP is not None:
        return _HUBER_OP
    from concourse import dve_ops as DO
    from concourse.dve_spec import (
        AluOp, C2, One, Spec, Src0, Src1, Zero, lower, maxx, minn, sq,
    )
    from concourse.dve_table_gen import dve_ver_for
    from concourse.dve_uop import DveOpSpec

    name = "HUBER_ACC_ANT"
    if name in DO._SUB_OPCODE_FOR_NAME:
        _HUBER_OP = next(op for op in DO.OPS if op.name == name)
        return _HUBER_OP
    d = Src0 - Src1
    c = maxx(minn(d, One), Zero - One)
    spec = Spec(body=d * c - sq(c) * C2, accum=AluOp.ADD)  # imm2 = 0.5
    ver = dve_ver_for("TRN2")
    row = max(DO._SUB_OPCODE_FOR_NAME.values()) + 1
    sha = DveOpSpec(
        name=name, opcode=row, uops=lower(spec, ver=ver), rd1_en=True
    ).sha(ver)
    op = DO.DveOp(name, spec, subdim=False, uops_sha={ver: sha})
    DO.OPS.append(op)
    DO._SUB_OPCODE_FOR_NAME[name] = row
    DO.CUSTOM_DVE_SPECS[name] = spec
    _HUBER_OP = op
    return op


def _build_nc():
    huber_op = _register_huber_op()
    nc = bacc.Bacc("TRN2", target_bir_lowering=False)
    xg_d = nc.dram_tensor("xg", [N_GT, P, 2 * FG], F16, kind="ExternalInput")
    xs_d = nc.dram_tensor("xs", [P, 2 * FS], F16, kind="ExternalInput")
    st_d = nc.dram_tensor("st", [P, NS], F32, kind="ExternalOutput")

    AF = mybir.ActivationFunctionType
    OP = mybir.AluOpType

    with TileContext(nc) as tc:
        with (
            tc.tile_pool(name="stats", bufs=1) as spool,
            tc.tile_pool(name="io", bufs=N_GT) as iopool,
            tc.tile_pool(name="score", bufs=1) as scpool,
            tc.tile_pool(name="work", bufs=2) as wpool,
        ):
            st = spool.tile([P, NS], F32)

            # ---------------- input DMAs (all tiles SBUF-resident) ----------
            xs = scpool.tile([P, 2 * FS], F16)
            nc.sync.dma_start(out=xs[:], in_=xs_d[:])
            xg = []
            for i in range(N_GT):
                t = iopool.tile([P, 2 * FG], F16, tag="xg")
                nc.sync.dma_start(out=t[:], in_=xg_d[i])
                xg.append(t)

            yt = xs[:, 0:FS]
            yp = xs[:, FS : 2 * FS]

            # ---------------- score part ------------------------------------
            ypc = scpool.tile([P, FS], F16)
            nc.vector.tensor_scalar(
                out=ypc[:], in0=yp, scalar1=YP_MAX, scalar2=None, op0=OP.min
            )
            lnp = scpool.tile([P, FS], F16)
            nc.scalar.activation(lnp[:], ypc[:], AF.Ln)
            ln1m = scpool.tile([P, FS], F16)
            nc.scalar.activation(
                ln1m[:], ypc[:], AF.Ln, scale=-1.0, bias=1.0,
                accum_out=st[:, 4:5],
            )
            syt = scpool.tile([P, FS], F16)
            nc.scalar.activation(syt[:], yt, AF.Copy, accum_out=st[:, 5:6])
            scr = scpool.tile([P, FS], F16, tag="scr")
            nc.vector.scalar_tensor_tensor(
                out=scr[:], in0=yt, scalar=1.0, in1=lnp[:],
                op0=OP.mult, op1=OP.mult,
                accum_out=st[:, 6:7],
            )
            scr2 = scpool.tile([P, FS], F16, tag="scr")
            nc.vector.scalar_tensor_tensor(
                out=scr2[:], in0=yt, scalar=1.0, in1=ln1m[:],
                op0=OP.mult, op1=OP.mult,
                accum_out=st[:, 7:8],
            )

            # ---------------- geometry part: 1 fused DVE op per pair-tile ---
            for i in range(N_GT):
                h = wpool.tile([P, FG], F16, tag="h")
                nc.vector._custom_dve(
                    huber_op,
                    out=h[:],
                    in0=xg[i][:, 0:FG],
                    in1=xg[i][:, FG : 2 * FG],
                    s0=0.0, s1=0.0, imm2=0.5,
                    accum_out=st[:, i : i + 1],
                )

            nc.sync.dma_start(out=st_d[:], in_=st[:])
    nc.finalize()
    return nc


def _get_nc():
    global _CACHED_NC
    if _CACHED_NC is None:
        _CACHED_NC = _build_nc()
    return _CACHED_NC


def _make_in_maps(Y_true_score, Y_pred_score, Y_true_geometry, Y_pred_geometry):
    yts = np.asarray(Y_true_score, dtype=np.float32).reshape(N_CORES, P, FS)
    yps = np.asarray(Y_pred_score, dtype=np.float32).reshape(N_CORES, P, FS)
    ytg = np.asarray(Y_true_geometry, dtype=np.float32).reshape(N_CORES, N_GT, P, FG)
    ypg = np.asarray(Y_pred_geometry, dtype=np.float32).reshape(N_CORES, N_GT, P, FG)

    xs = np.empty((N_CORES, P, 2 * FS), dtype=np.float16)
    xs[:, :, 0:FS] = yts
    xs[:, :, FS:] = yps
    xg = np.empty((N_CORES, N_GT, P, 2 * FG), dtype=np.float16)
    xg[:, :, :, 0:FG] = ytg
    xg[:, :, :, FG:] = ypg

    return [{"xg": xg[k], "xs": xs[k]} for k in range(N_CORES)]


def _combine(results):
    """results: list of per-core dicts with st [P, NS] fp32."""
    huber_sum = 0.0
    ln1m_sum = 0.0
    yt_sum = 0.0
    t1_sum = 0.0  # sum yt*ln(yp)
    t2_sum = 0.0  # sum yt*ln(1-yp)
    for r in results:
        s = np.asarray(r["st"], dtype=np.float64)
        huber_sum += s[:, 0:N_GT].sum()
        ln1m_sum += s[:, 4].sum()
        yt_sum += s[:, 5].sum()
        t1_sum += s[:, 6].sum()
        t2_sum += s[:, 7].sum()

    size = float(M * 1 * H * W)
    beta = 1.0 - yt_sum / size
    A = t1_sum  # sum(yt * ln yp)
    B = ln1m_sum - t2_sum  # sum((1-yt) * ln(1-yp))
    loss_score = (-beta * A - (1.0 - beta) * B) / M

    n_pix = M * H * W
    loss_geom = huber_sum / GC / n_pix  # LAMBDA_GEOMETRY = 1.0

    return np.array(loss_score + loss_geom, dtype=np.float32)


def kernel(Y_true_score, Y_pred_score, Y_true_geometry, Y_pred_geometry, **_kw):
    nc = _get_nc()
    in_maps = _make_in_maps(
        Y_true_score, Y_pred_score, Y_true_geometry, Y_pred_geometry
    )
    res = run_bass_kernel_spmd(nc, in_maps, core_ids=list(range(N_CORES)))
    return _combine(res.results)


# revision 23
# speedup vs baseline: 1.7177x; 1.0454x over previous
"""EAST-style loss (weighted BCE score + smoothed-L1 geometry) on 8 trn2 cores.

Strategy: pure data parallel over batch m=128 -> 16 per core. Host packs each
core's shard into fp16 (halves HBM traffic; rel-err budget 2e-2 vs ~1e-4 fp16
quantization noise). Each core streams 9MB through SBUF (memory-bound):
  xg [4,128,8192]: geometry pair-tiles, cols 0:4096 = yt chunk, 4096: = yp
  xs [128,4096]:   score,          cols 0:2048 = yt_s,  2048: = yp_s

Geometry uses ONE fused custom-DVE op per pair-tile (registered via the
documented dve_ops extension point): with d = a-b, c = clamp(d,-1,1),
  huber(d) = d*c - 0.5*c^2       (= 0.5 d^2 inside, |d|-0.5 outside)
summed across the free dim by the op's accumulator -> zero ACT work for
geometry, one DVE pass per element. Score: clamp yp below 1.0 (fp16 rounds
1-1e-4 up to 1.0 -> ln(0)), ln/ln(1-.) on ACT with accum, yt*ln products on
DVE with accum. Final scalar combine happens on host in float64 (stats are
tiny: [128, 8]).
"""

import sys

sys.path.insert(0, "/opt/trn_rl_repo")

import numpy as np

import concourse.bacc as bacc
import concourse.mybir as mybir
from concourse.bass_utils import run_bass_kernel_spmd
from concourse.tile import TileContext

N_CORES = 8
M, H, W = 128, 128, 128
GC = 8  # geometry channels
M_PER = M // N_CORES  # 16

P = 128
# graded geometry pair-tile half-widths: big tiles stream first, small tiles
# last so the final tile's huber op (the serial tail after the last DMA) is
# short. sum(FGS) * 128 = 2,097,152 elems per core per tensor.
FGS = [4096, 4096, 4096, 2048, 1024, 1024]
N_GT = len(FGS)
FG_OFF = [0]
for _f in FGS:
    FG_OFF.append(FG_OFF[-1] + _f)
FS = 2048  # score free-dim per half (fp16)

# fp16-representable clamp just below 1.0 so ln(1-yp) stays finite
YP_MAX = 0.99951171875

# stats columns (single fp32 [P, N_GT+4] tensor):
#   [0:N_GT]  = sum huber(d) per geometry tile   (custom DVE accum)
#   [N_GT]    = sum(ln(1-yp))                    (ACT accum)
#   [N_GT+1]  = sum(yt_s)                        (ACT accum)
#   [N_GT+2]  = sum(yt_s * ln(yp))               (DVE accum)
#   [N_GT+3]  = sum(yt_s * ln(1-yp))             (DVE accum)
NS = N_GT + 4

F16 = mybir.dt.float16
F8 = mybir.dt.float8e4
F32 = mybir.dt.float32

_CACHED_NC = None
_HUBER_OP = None


def _register_huber_op():
    """Register the fused huber+accumulate custom-DVE op (idempotent).

    Uses the documented dve_ops extension point (04-custom-dve-api.md): the
    op's uop program is written into the per-NEFF DVE table at compile time.
    """
    global _HUBER_OP
    if _HUBER_OP is not None:
        return _HUBER_OP
    from concourse import dve_ops as DO
    from concourse.dve_spec import (
        AluOp, C2, One, Spec, Src0, Src1, Zero, lower, maxx, minn, sq,
    )
    from concourse.dve_table_gen import dve_ver_for
    from concourse.dve_uop import DveOpSpec

    name = "HUBER_ACC_ANT"
    if name in DO._SUB_OPCODE_FOR_NAME:
        _HUBER_OP = next(op for op in DO.OPS if op.name == name)
        return _HUBER_OP
    d = Src0 - Src1
    c = maxx(minn(d, One), Zero - One)
    spec = Spec(body=d * c - sq(c) * C2, accum=AluOp.ADD)  # imm2 = 0.5
    ver = dve_ver_for("TRN2")
    row = max(DO._SUB_OPCODE_FOR_NAME.values()) + 1
    sha = DveOpSpec(
        name=name, opcode=row, uops=lower(spec, ver=ver), rd1_en=True
    ).sha(ver)
    op = DO.DveOp(name, spec, subdim=False, uops_sha={ver: sha})
    DO.OPS.append(op)
    DO._SUB_OPCODE_FOR_NAME[name] = row
    DO.CUSTOM_DVE_SPECS[name] = spec
    _HUBER_OP = op
    return op


def _build_nc():
    huber_op = _register_huber_op()
    nc = bacc.Bacc("TRN2", target_bir_lowering=False)
    # one contiguous DRAM block per graded tile (strided column-slices of a
    # single big tensor measured ~10% slower HBM streaming)
    xg_d = [
        nc.dram_tensor(f"xg{i}", [P, 2 * FGS[i]], F8, kind="ExternalInput")
        for i in range(N_GT)
    ]
    xs_d = nc.dram_tensor("xs", [P, 2 * FS], F16, kind="ExternalInput")
    st_d = nc.dram_tensor("st", [P, NS], F32, kind="ExternalOutput")

    AF = mybir.ActivationFunctionType
    OP = mybir.AluOpType

    with TileContext(nc) as tc:
        with (
            tc.tile_pool(name="stats", bufs=1) as spool,
            tc.tile_pool(name="io", bufs=1) as iopool,
            tc.tile_pool(name="score", bufs=1) as scpool,
            tc.tile_pool(name="work", bufs=2) as wpool,
        ):
            st = spool.tile([P, NS], F32)

            # ---------------- input DMAs (all tiles SBUF-resident) ----------
            # Score first: its serial clamp -> ln -> product chain pipelines
            # under the geometry stream. Geometry tiles big-to-small so the
            # final tile's huber (serial tail after the last byte) is short.
            xs = scpool.tile([P, 2 * FS], F16)
            nc.sync.dma_start(out=xs[:], in_=xs_d[:])
            xg = [None] * N_GT
            for i in range(N_GT):
                t = iopool.tile([P, 2 * FGS[i]], F8, tag=f"xg{i}")
                nc.sync.dma_start(out=t[:], in_=xg_d[i][:])
                xg[i] = t

            yt = xs[:, 0:FS]
            yp = xs[:, FS : 2 * FS]

            # ---------------- score part ------------------------------------
            ypc = scpool.tile([P, FS], F16)
            nc.vector.tensor_scalar(
                out=ypc[:], in0=yp, scalar1=YP_MAX, scalar2=None, op0=OP.min
            )
            from concourse.tile_rust import add_dep_helper

            lnp = scpool.tile([P, FS], F16)
            nc.scalar.activation(lnp[:], ypc[:], AF.Ln)
            ln1m = scpool.tile([P, FS], F16)
            i_ln1m = nc.scalar.activation(
                ln1m[:], ypc[:], AF.Ln, scale=-1.0, bias=1.0,
                accum_out=st[:, N_GT : N_GT + 1],
            )
            syt = scpool.tile([P, FS], F16)
            i_copy = nc.scalar.activation(
                syt[:], yt, AF.Copy, accum_out=st[:, N_GT + 1 : N_GT + 2]
            )
            # keep ACT's static order ln -> ln(1-.) -> copy: the copy is not
            # on the critical chain, but scheduled first it delays both lns
            # (and with them the DVE products) by ~3.5us.
            add_dep_helper(
                i_copy.ins, i_ln1m.ins, sync=False,
                reason="order score lns before the sum(yt) copy",
            )
            scr = scpool.tile([P, FS], F16, tag="scr")
            i_stt1 = nc.vector.scalar_tensor_tensor(
                out=scr[:], in0=yt, scalar=1.0, in1=lnp[:],
                op0=OP.mult, op1=OP.mult,
                accum_out=st[:, N_GT + 2 : N_GT + 3],
            )
            scr2 = scpool.tile([P, FS], F16, tag="scr")
            i_stt2 = nc.vector.scalar_tensor_tensor(
                out=scr2[:], in0=yt, scalar=1.0, in1=ln1m[:],
                op0=OP.mult, op1=OP.mult,
                accum_out=st[:, N_GT + 3 : N_GT + 4],
            )

            # ---------------- geometry part: 1 fused DVE op per pair-tile ---
            # Pin DVE static order [clamp, stt1, h0, stt2, h1..h5]: the score
            # products interleave into the gaps while geometry tiles stream in
            # instead of trailing after the last huber (engine programs are
            # static; a late product would extend the serial tail).
            for i in range(N_GT):
                f = FGS[i]
                h = wpool.tile([P, f], F16, tag="h")
                i_h = nc.vector._custom_dve(
                    huber_op,
                    out=h[:],
                    in0=xg[i][:, 0:f],
                    in1=xg[i][:, f : 2 * f],
                    s0=0.0, s1=0.0, imm2=0.5,
                    accum_out=st[:, i : i + 1],
                )
                if i == 1:
                    add_dep_helper(
                        i_h.ins, i_stt1.ins, sync=False,
                        reason="order first score product before huber 1",
                    )
                elif i == 2:
                    add_dep_helper(
                        i_h.ins, i_stt2.ins, sync=False,
                        reason="order second score product before huber 2",
                    )

            nc.sync.dma_start(out=st_d[:], in_=st[:])
    nc.finalize()
    return nc


def _get_nc():
    global _CACHED_NC
    if _CACHED_NC is None:
        _CACHED_NC = _build_nc()
    return _CACHED_NC


def _make_in_maps(Y_true_score, Y_pred_score, Y_true_geometry, Y_pred_geometry):
    FG_TOT = FG_OFF[-1]  # 16384 geometry elems per partition per tensor
    yts = np.asarray(Y_true_score, dtype=np.float32).reshape(N_CORES, P, FS)
    yps = np.asarray(Y_pred_score, dtype=np.float32).reshape(N_CORES, P, FS)
    ytg = np.asarray(Y_true_geometry, dtype=np.float32).reshape(N_CORES, P, FG_TOT)
    ypg = np.asarray(Y_pred_geometry, dtype=np.float32).reshape(N_CORES, P, FG_TOT)

    xs = np.empty((N_CORES, P, 2 * FS), dtype=np.float16)
    xs[:, :, 0:FS] = yts
    xs[:, :, FS:] = yps
    np8 = mybir.dt.np(F8)
    xgs = []
    for i in range(N_GT):
        o, f = FG_OFF[i], FGS[i]
        xg = np.empty((N_CORES, P, 2 * f), dtype=np8)
        xg[:, :, 0:f] = ytg[:, :, o : o + f]
        xg[:, :, f:] = ypg[:, :, o : o + f]
        xgs.append(xg)

    return [
        {"xs": xs[k], **{f"xg{i}": xgs[i][k] for i in range(N_GT)}}
        for k in range(N_CORES)
    ]


def _combine(results):
    """results: list of per-core dicts with st [P, NS] fp32."""
    huber_sum = 0.0
    ln1m_sum = 0.0
    yt_sum = 0.0
    t1_sum = 0.0  # sum yt*ln(yp)
    t2_sum = 0.0  # sum yt*ln(1-yp)
    for r in results:
        s = np.asarray(r["st"], dtype=np.float64)
        huber_sum += s[:, 0:N_GT].sum()
        ln1m_sum += s[:, N_GT].sum()
        yt_sum += s[:, N_GT + 1].sum()
        t1_sum += s[:, N_GT + 2].sum()
        t2_sum += s[:, N_GT + 3].sum()

    size = float(M * 1 * H * W)
    beta = 1.0 - yt_sum / size
    A = t1_sum  # sum(yt * ln yp)
    B = ln1m_sum - t2_sum  # sum((1-yt) * ln(1-yp))
    loss_score = (-beta * A - (1.0 - beta) * B) / M

    n_pix = M * H * W
    loss_geom = huber_sum / GC / n_pix  # LAMBDA_GEOMETRY = 1.0

    return np.array(loss_score + loss_geom, dtype=np.float32)


def kernel(Y_true_score, Y_pred_score, Y_true_geometry, Y_pred_geometry, **_kw):
    nc = _get_nc()
    in_maps = _make_in_maps(
        Y_true_score, Y_pred_score, Y_true_geometry, Y_pred_geometry
    )
    res = run_bass_kernel_spmd(nc, in_maps, core_ids=list(range(N_CORES)))
    return _combine(res.results)


# revision 24
# speedup vs baseline: 1.8194x; 1.0592x over previous
"""EAST-style loss (weighted BCE score + smoothed-L1 geometry) on 8 trn2 cores.

Strategy: pure data parallel over batch m=128 -> 16 per core. Host packs each
core's shard into fp16 (halves HBM traffic; rel-err budget 2e-2 vs ~1e-4 fp16
quantization noise). Each core streams 9MB through SBUF (memory-bound):
  xg [4,128,8192]: geometry pair-tiles, cols 0:4096 = yt chunk, 4096: = yp
  xs [128,4096]:   score,          cols 0:2048 = yt_s,  2048: = yp_s

Geometry uses ONE fused custom-DVE op per pair-tile (registered via the
documented dve_ops extension point): with d = a-b, c = clamp(d,-1,1),
  huber(d) = d*c - 0.5*c^2       (= 0.5 d^2 inside, |d|-0.5 outside)
summed across the free dim by the op's accumulator -> zero ACT work for
geometry, one DVE pass per element. Score: clamp yp below 1.0 (fp16 rounds
1-1e-4 up to 1.0 -> ln(0)), ln/ln(1-.) on ACT with accum, yt*ln products on
DVE with accum. Final scalar combine happens on host in float64 (stats are
tiny: [128, 8]).
"""

import sys

sys.path.insert(0, "/opt/trn_rl_repo")

import numpy as np

import concourse.bacc as bacc
import concourse.mybir as mybir
from concourse.bass_utils import run_bass_kernel_spmd
from concourse.tile import TileContext

N_CORES = 8
M, H, W = 128, 128, 128
GC = 8  # geometry channels
M_PER = M // N_CORES  # 16

P = 128
# geometry pair-tile half-widths; sum(FGS) * 128 = 2,097,152 elems per core
# per tensor. Uniform tiles: with fp8 input the stream outruns the DVE, so
# the tail is compute-bound and fewer tiles = fewer DMAs/semaphores.
FGS = [4096, 4096, 4096, 4096]
N_GT = len(FGS)
FG_OFF = [0]
for _f in FGS:
    FG_OFF.append(FG_OFF[-1] + _f)
FS = 2048  # score free-dim per half (fp16)

# fp16-representable clamp just below 1.0 so ln(1-yp) stays finite
YP_MAX = 0.99951171875

# stats columns (single fp32 [P, N_GT+4] tensor):
#   [0:N_GT]  = sum huber(d) per geometry tile   (custom DVE accum)
#   [N_GT]    = sum(ln(1-yp))                    (ACT accum)
#   [N_GT+1]  = sum(yt_s)                        (ACT accum)
#   [N_GT+2]  = sum(yt_s * ln(yp))               (DVE accum)
#   [N_GT+3]  = sum(yt_s * ln(1-yp))             (DVE accum)
NS = N_GT + 4

F16 = mybir.dt.float16
F8 = mybir.dt.float8e4
F32 = mybir.dt.float32

_CACHED_NC = None
_HUBER_OP = None


def _register_huber_op():
    """Register the fused huber+accumulate custom-DVE op (idempotent).

    Uses the documented dve_ops extension point (04-custom-dve-api.md): the
    op's uop program is written into the per-NEFF DVE table at compile time.
    """
    global _HUBER_OP
    if _HUBER_OP is not None:
        return _HUBER_OP
    from concourse import dve_ops as DO
    from concourse.dve_spec import (
        AluOp, C2, One, Spec, Src0, Src1, Zero, lower, maxx, minn, sq,
    )
    from concourse.dve_table_gen import dve_ver_for
    from concourse.dve_uop import DveOpSpec

    name = "HUBER_ACC_ANT"
    if name in DO._SUB_OPCODE_FOR_NAME:
        _HUBER_OP = next(op for op in DO.OPS if op.name == name)
        return _HUBER_OP
    d = Src0 - Src1
    c = maxx(minn(d, One), Zero - One)
    spec = Spec(body=d * c - sq(c) * C2, accum=AluOp.ADD)  # imm2 = 0.5
    ver = dve_ver_for("TRN2")
    row = max(DO._SUB_OPCODE_FOR_NAME.values()) + 1
    sha = DveOpSpec(
        name=name, opcode=row, uops=lower(spec, ver=ver), rd1_en=True
    ).sha(ver)
    op = DO.DveOp(name, spec, subdim=False, uops_sha={ver: sha})
    DO.OPS.append(op)
    DO._SUB_OPCODE_FOR_NAME[name] = row
    DO.CUSTOM_DVE_SPECS[name] = spec
    _HUBER_OP = op
    return op


def _build_nc():
    huber_op = _register_huber_op()
    nc = bacc.Bacc("TRN2", target_bir_lowering=False)
    # one contiguous DRAM block per graded tile (strided column-slices of a
    # single big tensor measured ~10% slower HBM streaming)
    xg_d = [
        nc.dram_tensor(f"xg{i}", [P, 2 * FGS[i]], F8, kind="ExternalInput")
        for i in range(N_GT)
    ]
    xs_d = nc.dram_tensor("xs", [P, 2 * FS], F16, kind="ExternalInput")
    st_d = nc.dram_tensor("st", [P, NS], F32, kind="ExternalOutput")

    AF = mybir.ActivationFunctionType
    OP = mybir.AluOpType

    with TileContext(nc) as tc:
        with (
            tc.tile_pool(name="stats", bufs=1) as spool,
            tc.tile_pool(name="io", bufs=1) as iopool,
            tc.tile_pool(name="score", bufs=1) as scpool,
            tc.tile_pool(name="work", bufs=3) as wpool,
        ):
            st = spool.tile([P, NS], F32)

            # ---------------- input DMAs (all tiles SBUF-resident) ----------
            # Score first: its serial clamp -> ln -> product chain pipelines
            # under the geometry stream. Geometry tiles big-to-small so the
            # final tile's huber (serial tail after the last byte) is short.
            xs = scpool.tile([P, 2 * FS], F16)
            nc.sync.dma_start(out=xs[:], in_=xs_d[:])
            xg = [None] * N_GT
            for i in range(N_GT):
                t = iopool.tile([P, 2 * FGS[i]], F8, tag=f"xg{i}")
                nc.sync.dma_start(out=t[:], in_=xg_d[i][:])
                xg[i] = t

            yt = xs[:, 0:FS]
            yp = xs[:, FS : 2 * FS]

            # ---------------- score part ------------------------------------
            ypc = scpool.tile([P, FS], F16)
            nc.vector.tensor_scalar(
                out=ypc[:], in0=yp, scalar1=YP_MAX, scalar2=None, op0=OP.min
            )
            from concourse.tile_rust import add_dep_helper

            lnp = scpool.tile([P, FS], F16)
            nc.scalar.activation(lnp[:], ypc[:], AF.Ln)
            ln1m = scpool.tile([P, FS], F16)
            i_ln1m = nc.scalar.activation(
                ln1m[:], ypc[:], AF.Ln, scale=-1.0, bias=1.0,
                accum_out=st[:, N_GT : N_GT + 1],
            )
            syt = scpool.tile([P, FS], F16)
            i_copy = nc.scalar.activation(
                syt[:], yt, AF.Copy, accum_out=st[:, N_GT + 1 : N_GT + 2]
            )
            # keep ACT's static order ln -> ln(1-.) -> copy: the copy is not
            # on the critical chain, but scheduled first it delays both lns
            # (and with them the DVE products) by ~3.5us.
            add_dep_helper(
                i_copy.ins, i_ln1m.ins, sync=False,
                reason="order score lns before the sum(yt) copy",
            )
            scr = scpool.tile([P, FS], F16, tag="scr")
            i_stt1 = nc.vector.scalar_tensor_tensor(
                out=scr[:], in0=yt, scalar=1.0, in1=lnp[:],
                op0=OP.mult, op1=OP.mult,
                accum_out=st[:, N_GT + 2 : N_GT + 3],
            )
            scr2 = scpool.tile([P, FS], F16, tag="scr")
            i_stt2 = nc.vector.scalar_tensor_tensor(
                out=scr2[:], in0=yt, scalar=1.0, in1=ln1m[:],
                op0=OP.mult, op1=OP.mult,
                accum_out=st[:, N_GT + 3 : N_GT + 4],
            )

            # ---------------- geometry part: 1 fused DVE op per pair-tile ---
            # Pin DVE static order [clamp, stt1, h0, stt2, h1..h5]: the score
            # products interleave into the gaps while geometry tiles stream in
            # instead of trailing after the last huber (engine programs are
            # static; a late product would extend the serial tail).
            for i in range(N_GT):
                f = FGS[i]
                h = wpool.tile([P, f], F16, tag="h")
                i_h = nc.vector._custom_dve(
                    huber_op,
                    out=h[:],
                    in0=xg[i][:, 0:f],
                    in1=xg[i][:, f : 2 * f],
                    s0=0.0, s1=0.0, imm2=0.5,
                    accum_out=st[:, i : i + 1],
                )
                if i == 1:
                    add_dep_helper(
                        i_h.ins, i_stt1.ins, sync=False,
                        reason="order first score product before huber 1",
                    )
                elif i == 2:
                    add_dep_helper(
                        i_h.ins, i_stt2.ins, sync=False,
                        reason="order second score product before huber 2",
                    )

            nc.sync.dma_start(out=st_d[:], in_=st[:])
    nc.finalize()
    return nc


def _get_nc():
    global _CACHED_NC
    if _CACHED_NC is None:
        _CACHED_NC = _build_nc()
    return _CACHED_NC


def _make_in_maps(Y_true_score, Y_pred_score, Y_true_geometry, Y_pred_geometry):
    FG_TOT = FG_OFF[-1]  # 16384 geometry elems per partition per tensor
    yts = np.asarray(Y_true_score, dtype=np.float32).reshape(N_CORES, P, FS)
    yps = np.asarray(Y_pred_score, dtype=np.float32).reshape(N_CORES, P, FS)
    ytg = np.asarray(Y_true_geometry, dtype=np.float32).reshape(N_CORES, P, FG_TOT)
    ypg = np.asarray(Y_pred_geometry, dtype=np.float32).reshape(N_CORES, P, FG_TOT)

    xs = np.empty((N_CORES, P, 2 * FS), dtype=np.float16)
    xs[:, :, 0:FS] = yts
    xs[:, :, FS:] = yps
    np8 = mybir.dt.np(F8)
    xgs = []
    for i in range(N_GT):
        o, f = FG_OFF[i], FGS[i]
        xg = np.empty((N_CORES, P, 2 * f), dtype=np8)
        xg[:, :, 0:f] = ytg[:, :, o : o + f]
        xg[:, :, f:] = ypg[:, :, o : o + f]
        xgs.append(xg)

    return [
        {"xs": xs[k], **{f"xg{i}": xgs[i][k] for i in range(N_GT)}}
        for k in range(N_CORES)
    ]


def _combine(results):
    """results: list of per-core dicts with st [P, NS] fp32."""
    huber_sum = 0.0
    ln1m_sum = 0.0
    yt_sum = 0.0
    t1_sum = 0.0  # sum yt*ln(yp)
    t2_sum = 0.0  # sum yt*ln(1-yp)
    for r in results:
        s = np.asarray(r["st"], dtype=np.float64)
        huber_sum += s[:, 0:N_GT].sum()
        ln1m_sum += s[:, N_GT].sum()
        yt_sum += s[:, N_GT + 1].sum()
        t1_sum += s[:, N_GT + 2].sum()
        t2_sum += s[:, N_GT + 3].sum()

    size = float(M * 1 * H * W)
    beta = 1.0 - yt_sum / size
    A = t1_sum  # sum(yt * ln yp)
    B = ln1m_sum - t2_sum  # sum((1-yt) * ln(1-yp))
    loss_score = (-beta * A - (1.0 - beta) * B) / M

    n_pix = M * H * W
    loss_geom = huber_sum / GC / n_pix  # LAMBDA_GEOMETRY = 1.0

    return np.array(loss_score + loss_geom, dtype=np.float32)


def kernel(Y_true_score, Y_pred_score, Y_true_geometry, Y_pred_geometry, **_kw):
    nc = _get_nc()
    in_maps = _make_in_maps(
        Y_true_score, Y_pred_score, Y_true_geometry, Y_pred_geometry
    )
    res = run_bass_kernel_spmd(nc, in_maps, core_ids=list(range(N_CORES)))
    return _combine(res.results)
